# revision 1
# baseline (speedup 1.0000x reference)
"""Distributed 1-NN style-bank retrieval on 8 Trainium2 NeuronCores.

reference semantics:
    cs  = content.reshape(64, 524288), L2-normalized rows
    ct  = bank_content.reshape(524288, 256), L2-normalized cols
    idx = argmax(cs @ ct, axis=1);  out = bank_style[idx]

Strategy: shard the contraction axis D=524288 across the 8 cores (each core
reads every input byte exactly once — I/O optimal). Each core computes, in
bf16 with f32 PSUM accumulation:
  - partial dot[64, 256] = cs_shard @ ct_shard  (query normalization cancels
    in the argmax, so it is skipped entirely)
  - partial column sum-of-squares of ct_shard (for the bank-side norms)
The host sums the 8 tiny partials, forms sim = dot/sqrt(ssq), takes the
argmax, and exactly re-ranks (f64) any candidate within a safety margin of
the winner — the margin is ~8x the measured bf16 perturbation, so the
low-precision pass can never silently flip a near-tie.
"""

import os

import numpy as np
import ml_dtypes

B, D, M, S = 64, 524288, 256, 2048
NCORES = 8
DSH = D // NCORES          # 65536 contraction rows per core
KT = DSH // 128            # 512 k-tiles of 128
G = 32                     # k-tiles per DMA block
NBLK = KT // G             # 16
BF16 = ml_dtypes.bfloat16

# |bf16 sim - exact sim| measured at 1.3e-5 (cosine units) on randn inputs of
# this shape; re-rank everything within 8x that of the bf16 winner.
RERANK_MARGIN = 1e-4

_CACHED_NC = None


def _build_nc():
    import concourse.bacc as bacc
    import concourse.mybir as mybir
    from concourse import tile

    nc = bacc.Bacc("TRN2", target_bir_lowering=False, debug=False,
                   num_devices=NCORES)
    qT = nc.dram_tensor("qT", [128, KT, B], mybir.dt.bfloat16,
                        kind="ExternalInput")
    bank = nc.dram_tensor("bank", [DSH, M], mybir.dt.bfloat16,
                          kind="ExternalInput")
    dot_out = nc.dram_tensor("dot_out", [128, M], mybir.dt.float32,
                             kind="ExternalOutput")
    ssq_out = nc.dram_tensor("ssq_out", [1, 2 * M], mybir.dt.float32,
                             kind="ExternalOutput")

    with tile.TileContext(nc) as tc:
        with tc.tile_pool(name="lhs", bufs=3) as plhs, \
             tc.tile_pool(name="rhs", bufs=3) as prhs, \
             tc.tile_pool(name="sq", bufs=2) as psq, \
             tc.tile_pool(name="misc", bufs=1) as pmisc, \
             tc.tile_pool(name="psum", bufs=1, space="PSUM") as pps:
            ones = pmisc.tile([128, 1], mybir.dt.bfloat16)
            nc.any.memset(ones[:], 1.0)
            ps_dot = pps.tile([128, M], mybir.dt.float32)
            ps_ssq = pps.tile([1, 2 * M], mybir.dt.float32)
            # view bank rows as (blk, g, p): k-tile t = blk*G+g, partition p
            bank_t = bank.rearrange("(blk g p) n -> blk p g n", g=G, p=128)
            for blk in range(NBLK):
                lt = plhs.tile([128, G, B], mybir.dt.bfloat16)
                nc.sync.dma_start(lt[:], qT[:, blk * G:(blk + 1) * G, :])
                rt = prhs.tile([128, G, M], mybir.dt.bfloat16)
                nc.sync.dma_start(rt[:], bank_t[blk])
                sq = psq.tile([128, G, M], mybir.dt.bfloat16)
                nc.vector.tensor_mul(sq[:], rt[:], rt[:])
                for j in range(G):
                    g = blk * G + j
                    # even k-tiles accumulate into PSUM partitions 0:64,
                    # odd into 64:128 (PE col-group packing — the two run
                    # concurrently); host adds the halves.
                    half = 64 * (g % 2)
                    nc.tensor.matmul(
                        ps_dot[half:half + 64, :],
                        lt[:, j, :],
                        rt[:, j, :],
                        start=(g < 2),
                        stop=(g >= KT - 2),
                    )
                for jj in range(G // 2):
                    gg = blk * (G // 2) + jj
                    nc.tensor.matmul(
                        ps_ssq[:, :],
                        ones[:],
                        sq[:, 2 * jj:2 * jj + 2, :],
                        start=(gg == 0),
                        stop=(gg == KT // 2 - 1),
                    )
            dot_sb = pmisc.tile([128, M], mybir.dt.float32)
            nc.scalar.copy(dot_sb[:], ps_dot[:])
            ssq_sb = pmisc.tile([1, 2 * M], mybir.dt.float32)
            nc.vector.tensor_copy(ssq_sb[:], ps_ssq[:])
            nc.sync.dma_start(dot_out[:], dot_sb[:])
            nc.sync.dma_start(ssq_out[:], ssq_sb[:])
    nc.compile()
    return nc


def _get_nc():
    global _CACHED_NC
    if _CACHED_NC is None:
        _CACHED_NC = _build_nc()
    return _CACHED_NC


def _make_qT(cs, lo):
    """[128, KT, B] bf16 with qT[p, t, b] = cs[b, lo + t*128 + p]."""
    csT = np.empty((DSH, B), BF16)
    BLK = 4096  # 64 x 4096 x 4B = 1 MiB working set per block
    sub = cs[:, lo:lo + DSH]
    for j in range(0, DSH, BLK):
        csT[j:j + BLK] = sub[:, j:j + BLK].T
    return np.ascontiguousarray(csT.reshape(KT, 128, B).transpose(1, 0, 2))


def _install_ntff_hook():
    """Register the axon NTFF profile hook missing from this image's antenv
    (profiling path only — used when BASSKNN_TRACE=1)."""
    import contextlib
    import ctypes
    import sys
    import types

    if "antenv.axon_hooks" in sys.modules:
        return
    lib = ctypes.CDLL("/opt/axon/libaxon_pjrt.so")
    lib.axon_start_nrt_profile.argtypes = [ctypes.POINTER(ctypes.c_int64),
                                           ctypes.c_size_t]
    lib.axon_start_nrt_profile.restype = ctypes.c_int64
    lib.axon_stop_nrt_profile.argtypes = [ctypes.c_char_p]
    lib.axon_stop_nrt_profile.restype = ctypes.c_int64

    @contextlib.contextmanager
    def _hook(output_dir, device_ids):
        import jax

        jax.devices()
        if device_ids:
            ids = (ctypes.c_int64 * len(device_ids))(*device_ids)
            rc = lib.axon_start_nrt_profile(ids, len(device_ids))
        else:
            rc = lib.axon_start_nrt_profile(None, 0)
        if rc != 0:
            raise RuntimeError(f"axon_start_nrt_profile rc={rc}")
        try:
            yield
        finally:
            n = lib.axon_stop_nrt_profile(str(output_dir).encode())
            print(f"ntff profile: {n} file(s) -> {output_dir}", file=sys.stderr)

    mod = types.ModuleType("antenv.axon_hooks")
    mod.get_axon_ntff_profile_hook = lambda: _hook
    sys.modules["antenv.axon_hooks"] = mod
    import concourse.bass_utils as bass_utils

    bass_utils.upload_artifacts = lambda tmpdir: "local://" + tmpdir


def kernel(content, bank_content, bank_style):
    from concourse.bass_utils import run_bass_kernel_spmd

    content = np.ascontiguousarray(content, dtype=np.float32)
    bank_content = np.ascontiguousarray(bank_content, dtype=np.float32)
    bank_style = np.asarray(bank_style)
    cs = content.reshape(B, D)
    ct = bank_content.reshape(D, M)  # raw row-major reshape, NOT a transpose

    in_maps = []
    for c in range(NCORES):
        lo = c * DSH
        in_maps.append({
            "qT": _make_qT(cs, lo),
            "bank": ct[lo:lo + DSH].astype(BF16),
        })

    nc = _get_nc()
    trace = bool(os.environ.get("BASSKNN_TRACE"))
    kwargs = {}
    if trace:
        _install_ntff_hook()
        kwargs = {"trace": True}
    res = run_bass_kernel_spmd(nc, in_maps, list(range(NCORES)), **kwargs)
    if trace:
        print(f"HW exec time: {res.exec_time_ns} ns")

    dot = np.zeros((B, M), np.float64)
    ssq = np.zeros((M,), np.float64)
    for c in range(NCORES):
        d = res.results[c]["dot_out"].astype(np.float64)
        dot += d[0:64] + d[64:128]
        s = res.results[c]["ssq_out"][0].astype(np.float64)
        ssq += s[:M] + s[M:]
    sim = dot / np.sqrt(ssq)[None, :]  # = cosine * ||cs_b||, per row b

    idx = sim.argmax(axis=1)
    # Exact re-rank of near-ties: any m whose bf16 sim is within
    # RERANK_MARGIN (cosine units) of the row max could be the true winner.
    row_norms = np.sqrt(np.einsum("bd,bd->b", cs, cs, dtype=np.float64))
    col_cache = {}
    for b in range(B):
        thr = RERANK_MARGIN * row_norms[b]
        cands = np.nonzero(sim[b] >= sim[b, idx[b]] - thr)[0]
        if len(cands) <= 1:
            continue
        row = cs[b].astype(np.float64)
        best_m, best_v = -1, -np.inf
        for m in sorted(int(x) for x in cands):
            if m not in col_cache:
                colf = ct[:, m].astype(np.float64)
                col_cache[m] = (colf, np.sqrt(colf @ colf))
            colf, nrm = col_cache[m]
            v = (row @ colf) / nrm
            if v > best_v:  # strict '>' keeps the lowest index on exact ties
                best_v, best_m = v, m
        idx[b] = best_m
    return bank_style[idx]


# revision 3
# speedup vs baseline: 1.0987x; 1.0987x over previous
"""Distributed 1-NN style-bank retrieval on 8 Trainium2 NeuronCores.

reference semantics:
    cs  = content.reshape(64, 524288), L2-normalized rows
    ct  = bank_content.reshape(524288, 256), L2-normalized cols
    idx = argmax(cs @ ct, axis=1);  out = bank_style[idx]

Strategy: shard the contraction axis D=524288 across the 8 cores (each core
reads every input byte exactly once — I/O optimal). Each core computes, in
bf16 with f32 PSUM accumulation:
  - partial dot[64, 256] = cs_shard @ ct_shard  (query normalization cancels
    in the argmax, so it is skipped entirely)
  - partial column sum-of-squares of ct_shard (for the bank-side norms)
The host sums the 8 tiny partials, forms sim = dot/sqrt(ssq), takes the
argmax, and exactly re-ranks (f64) any candidate within a safety margin of
the winner — the margin is ~8x the measured bf16 perturbation, so the
low-precision pass can never silently flip a near-tie.
"""

import os

import numpy as np
import ml_dtypes

B, D, M, S = 64, 524288, 256, 2048
NCORES = 8
DSH = D // NCORES          # 65536 contraction rows per core
KT = DSH // 128            # 512 k-tiles of 128
G = 32                     # k-tiles per DMA block
NBLK = KT // G             # 16
BF16 = ml_dtypes.bfloat16

# |bf16 sim - exact sim| measured at 1.3e-5 (cosine units) on randn inputs of
# this shape; re-rank everything within 8x that of the bf16 winner.
RERANK_MARGIN = 1e-4

_CACHED_NC = None


def _build_nc():
    import concourse.bacc as bacc
    import concourse.mybir as mybir
    from concourse import tile

    nc = bacc.Bacc("TRN2", target_bir_lowering=False, debug=False,
                   num_devices=NCORES)
    qT = nc.dram_tensor("qT", [128, KT, B], mybir.dt.bfloat16,
                        kind="ExternalInput")
    bank = nc.dram_tensor("bank", [128, KT, M], mybir.dt.bfloat16,
                          kind="ExternalInput")
    dot_out = nc.dram_tensor("dot_out", [128, M], mybir.dt.float32,
                             kind="ExternalOutput")
    ssq_out = nc.dram_tensor("ssq_out", [1, 2 * M], mybir.dt.float32,
                             kind="ExternalOutput")

    with tile.TileContext(nc) as tc:
        with tc.tile_pool(name="lhs", bufs=1) as plhs, \
             tc.tile_pool(name="rhs", bufs=3) as prhs, \
             tc.tile_pool(name="sq", bufs=2) as psq, \
             tc.tile_pool(name="misc", bufs=1) as pmisc, \
             tc.tile_pool(name="psum", bufs=1, space="PSUM") as pps:
            ones = pmisc.tile([128, 1], mybir.dt.bfloat16)
            nc.any.memset(ones[:], 1.0)
            ps_dot = pps.tile([128, M], mybir.dt.float32)
            ps_ssq = pps.tile([1, 2 * M], mybir.dt.float32)
            # all 512 query k-tiles stay resident (64 KiB/partition)
            lt = plhs.tile([128, KT, B], mybir.dt.bfloat16)
            nc.sync.dma_start(lt[:], qT[:])
            for blk in range(NBLK):
                rt = prhs.tile([128, G, M], mybir.dt.bfloat16)
                nc.sync.dma_start(rt[:], bank[:, blk * G:(blk + 1) * G, :])
                sq = psq.tile([128, G, M], mybir.dt.bfloat16)
                nc.vector.tensor_mul(sq[:], rt[:], rt[:])
                for j in range(G):
                    g = blk * G + j
                    # even k-tiles accumulate into PSUM partitions 0:64,
                    # odd into 64:128 (PE col-group packing — the two run
                    # concurrently); host adds the halves.
                    half = 64 * (g % 2)
                    nc.tensor.matmul(
                        ps_dot[half:half + 64, :],
                        lt[:, g, :],
                        rt[:, j, :],
                        start=(g < 2),
                        stop=(g >= KT - 2),
                    )
                for jj in range(G // 2):
                    gg = blk * (G // 2) + jj
                    nc.tensor.matmul(
                        ps_ssq[:, :],
                        ones[:],
                        sq[:, 2 * jj:2 * jj + 2, :],
                        start=(gg == 0),
                        stop=(gg == KT // 2 - 1),
                    )
            dot_sb = pmisc.tile([128, M], mybir.dt.float32)
            nc.scalar.copy(dot_sb[:], ps_dot[:])
            ssq_sb = pmisc.tile([1, 2 * M], mybir.dt.float32)
            nc.vector.tensor_copy(ssq_sb[:], ps_ssq[:])
            nc.sync.dma_start(dot_out[:], dot_sb[:])
            nc.sync.dma_start(ssq_out[:], ssq_sb[:])
    nc.compile()
    return nc


def _get_nc():
    global _CACHED_NC
    if _CACHED_NC is None:
        _CACHED_NC = _build_nc()
    return _CACHED_NC


def _make_qT(cs, lo):
    """[128, KT, B] bf16 with qT[p, t, b] = cs[b, lo + t*128 + p]."""
    csT = np.empty((DSH, B), BF16)
    BLK = 4096  # 64 x 4096 x 4B = 1 MiB working set per block
    sub = cs[:, lo:lo + DSH]
    for j in range(0, DSH, BLK):
        csT[j:j + BLK] = sub[:, j:j + BLK].T
    return np.ascontiguousarray(csT.reshape(KT, 128, B).transpose(1, 0, 2))


def _install_ntff_hook():
    """Register the axon NTFF profile hook missing from this image's antenv
    (profiling path only — used when BASSKNN_TRACE=1)."""
    import contextlib
    import ctypes
    import sys
    import types

    if "antenv.axon_hooks" in sys.modules:
        return
    lib = ctypes.CDLL("/opt/axon/libaxon_pjrt.so")
    lib.axon_start_nrt_profile.argtypes = [ctypes.POINTER(ctypes.c_int64),
                                           ctypes.c_size_t]
    lib.axon_start_nrt_profile.restype = ctypes.c_int64
    lib.axon_stop_nrt_profile.argtypes = [ctypes.c_char_p]
    lib.axon_stop_nrt_profile.restype = ctypes.c_int64

    @contextlib.contextmanager
    def _hook(output_dir, device_ids):
        import jax

        jax.devices()
        if device_ids:
            ids = (ctypes.c_int64 * len(device_ids))(*device_ids)
            rc = lib.axon_start_nrt_profile(ids, len(device_ids))
        else:
            rc = lib.axon_start_nrt_profile(None, 0)
        if rc != 0:
            raise RuntimeError(f"axon_start_nrt_profile rc={rc}")
        try:
            yield
        finally:
            n = lib.axon_stop_nrt_profile(str(output_dir).encode())
            print(f"ntff profile: {n} file(s) -> {output_dir}", file=sys.stderr)

    mod = types.ModuleType("antenv.axon_hooks")
    mod.get_axon_ntff_profile_hook = lambda: _hook
    sys.modules["antenv.axon_hooks"] = mod
    import concourse.bass_utils as bass_utils

    bass_utils.upload_artifacts = lambda tmpdir: "local://" + tmpdir


def kernel(content, bank_content, bank_style):
    from concourse.bass_utils import run_bass_kernel_spmd

    content = np.ascontiguousarray(content, dtype=np.float32)
    bank_content = np.ascontiguousarray(bank_content, dtype=np.float32)
    bank_style = np.asarray(bank_style)
    cs = content.reshape(B, D)
    ct = bank_content.reshape(D, M)  # raw row-major reshape, NOT a transpose

    in_maps = []
    for c in range(NCORES):
        lo = c * DSH
        bank_pm = np.ascontiguousarray(
            ct[lo:lo + DSH].reshape(KT, 128, M).transpose(1, 0, 2).astype(BF16))
        in_maps.append({
            "qT": _make_qT(cs, lo),
            "bank": bank_pm,
        })

    nc = _get_nc()
    trace = bool(os.environ.get("BASSKNN_TRACE"))
    kwargs = {}
    if trace:
        _install_ntff_hook()
        kwargs = {"trace": True}
    res = run_bass_kernel_spmd(nc, in_maps, list(range(NCORES)), **kwargs)
    if trace:
        print(f"HW exec time: {res.exec_time_ns} ns")

    dot = np.zeros((B, M), np.float64)
    ssq = np.zeros((M,), np.float64)
    for c in range(NCORES):
        d = res.results[c]["dot_out"].astype(np.float64)
        dot += d[0:64] + d[64:128]
        s = res.results[c]["ssq_out"][0].astype(np.float64)
        ssq += s[:M] + s[M:]
    sim = dot / np.sqrt(ssq)[None, :]  # = cosine * ||cs_b||, per row b

    idx = sim.argmax(axis=1)
    # Exact re-rank of near-ties: any m whose bf16 sim is within
    # RERANK_MARGIN (cosine units) of the row max could be the true winner.
    row_norms = np.sqrt(np.einsum("bd,bd->b", cs, cs, dtype=np.float64))
    col_cache = {}
    for b in range(B):
        thr = RERANK_MARGIN * row_norms[b]
        cands = np.nonzero(sim[b] >= sim[b, idx[b]] - thr)[0]
        if len(cands) <= 1:
            continue
        row = cs[b].astype(np.float64)
        best_m, best_v = -1, -np.inf
        for m in sorted(int(x) for x in cands):
            if m not in col_cache:
                colf = ct[:, m].astype(np.float64)
                col_cache[m] = (colf, np.sqrt(colf @ colf))
            colf, nrm = col_cache[m]
            v = (row @ colf) / nrm
            if v > best_v:  # strict '>' keeps the lowest index on exact ties
                best_v, best_m = v, m
        idx[b] = best_m
    return bank_style[idx]


# revision 4
# speedup vs baseline: 1.1495x; 1.0463x over previous
"""Distributed 1-NN style-bank retrieval on 8 Trainium2 NeuronCores.

reference semantics:
    cs  = content.reshape(64, 524288), L2-normalized rows
    ct  = bank_content.reshape(524288, 256), L2-normalized cols
    idx = argmax(cs @ ct, axis=1);  out = bank_style[idx]

Strategy: shard the contraction axis D=524288 across the 8 cores (each core
reads every input byte exactly once — I/O optimal). Each core computes, in
bf16 with f32 PSUM accumulation:
  - partial dot[64, 256] = cs_shard @ ct_shard  (query normalization cancels
    in the argmax, so it is skipped entirely)
  - partial column sum-of-squares of ct_shard (for the bank-side norms)
The host sums the 8 tiny partials, forms sim = dot/sqrt(ssq), takes the
argmax, and exactly re-ranks (f64) any candidate within a safety margin of
the winner — the margin is ~8x the measured bf16 perturbation, so the
low-precision pass can never silently flip a near-tie.
"""

import os

import numpy as np
import ml_dtypes

B, D, M, S = 64, 524288, 256, 2048
NCORES = 8
DSH = D // NCORES          # 65536 contraction rows per core
KT = DSH // 128            # 512 k-tiles of 128
G = 32                     # k-tiles per DMA block
NBLK = KT // G             # 16
BF16 = ml_dtypes.bfloat16

# |bf16 sim - exact sim| measured at 1.3e-5 (cosine units) on randn inputs of
# this shape; re-rank everything within 8x that of the bf16 winner.
RERANK_MARGIN = 1e-4

_CACHED_NC = None


def _build_nc():
    import concourse.bacc as bacc
    import concourse.mybir as mybir
    from concourse import tile

    nc = bacc.Bacc("TRN2", target_bir_lowering=False, debug=False,
                   num_devices=NCORES)
    qT = nc.dram_tensor("qT", [128, KT, B], mybir.dt.bfloat16,
                        kind="ExternalInput")
    bank = nc.dram_tensor("bank", [128, KT, M], mybir.dt.bfloat16,
                          kind="ExternalInput")
    dot_out = nc.dram_tensor("dot_out", [128, M], mybir.dt.float32,
                             kind="ExternalOutput")
    ssq_out = nc.dram_tensor("ssq_out", [1, 2 * M], mybir.dt.float32,
                             kind="ExternalOutput")

    with tile.TileContext(nc) as tc:
        with tc.tile_pool(name="lhs", bufs=1) as plhs, \
             tc.tile_pool(name="rhs", bufs=4) as prhs, \
             tc.tile_pool(name="sq", bufs=2) as psq, \
             tc.tile_pool(name="misc", bufs=1) as pmisc, \
             tc.tile_pool(name="psum", bufs=1, space="PSUM") as pps:
            ones = pmisc.tile([128, 1], mybir.dt.bfloat16)
            nc.any.memset(ones[:], 1.0)
            ps_dot = pps.tile([128, M], mybir.dt.float32)
            ps_ssq = pps.tile([1, 2 * M], mybir.dt.float32)
            # all 512 query k-tiles stay resident (64 KiB/partition); streamed
            # on the ACT HWDGE ring in chunks so the SP ring (bank stream)
            # isn't blocked behind one 8 MiB transfer.
            lt = plhs.tile([128, KT, B], mybir.dt.bfloat16)
            QCH = KT // 4
            for q in range(4):
                nc.scalar.dma_start(lt[:, q * QCH:(q + 1) * QCH, :],
                                    qT[:, q * QCH:(q + 1) * QCH, :])
            for blk in range(NBLK):
                rt = prhs.tile([128, G, M], mybir.dt.bfloat16)
                nc.sync.dma_start(rt[:], bank[:, blk * G:(blk + 1) * G, :])
                sq = psq.tile([128, G, M], mybir.dt.bfloat16)
                nc.vector.tensor_mul(sq[:], rt[:], rt[:])
                for j in range(G):
                    g = blk * G + j
                    # even k-tiles accumulate into PSUM partitions 0:64,
                    # odd into 64:128 (PE col-group packing — the two run
                    # concurrently); host adds the halves.
                    half = 64 * (g % 2)
                    nc.tensor.matmul(
                        ps_dot[half:half + 64, :],
                        lt[:, g, :],
                        rt[:, j, :],
                        start=(g < 2),
                        stop=(g >= KT - 2),
                    )
                for jj in range(G // 2):
                    gg = blk * (G // 2) + jj
                    nc.tensor.matmul(
                        ps_ssq[:, :],
                        ones[:],
                        sq[:, 2 * jj:2 * jj + 2, :],
                        start=(gg == 0),
                        stop=(gg == KT // 2 - 1),
                    )
            dot_sb = pmisc.tile([128, M], mybir.dt.float32)
            nc.scalar.copy(dot_sb[:], ps_dot[:])
            ssq_sb = pmisc.tile([1, 2 * M], mybir.dt.float32)
            nc.vector.tensor_copy(ssq_sb[:], ps_ssq[:])
            nc.sync.dma_start(dot_out[:], dot_sb[:])
            nc.sync.dma_start(ssq_out[:], ssq_sb[:])
    nc.compile()
    return nc


def _get_nc():
    global _CACHED_NC
    if _CACHED_NC is None:
        _CACHED_NC = _build_nc()
    return _CACHED_NC


def _make_qT(cs, lo):
    """[128, KT, B] bf16 with qT[p, t, b] = cs[b, lo + t*128 + p]."""
    csT = np.empty((DSH, B), BF16)
    BLK = 4096  # 64 x 4096 x 4B = 1 MiB working set per block
    sub = cs[:, lo:lo + DSH]
    for j in range(0, DSH, BLK):
        csT[j:j + BLK] = sub[:, j:j + BLK].T
    return np.ascontiguousarray(csT.reshape(KT, 128, B).transpose(1, 0, 2))


def _install_ntff_hook():
    """Register the axon NTFF profile hook missing from this image's antenv
    (profiling path only — used when BASSKNN_TRACE=1)."""
    import contextlib
    import ctypes
    import sys
    import types

    if "antenv.axon_hooks" in sys.modules:
        return
    lib = ctypes.CDLL("/opt/axon/libaxon_pjrt.so")
    lib.axon_start_nrt_profile.argtypes = [ctypes.POINTER(ctypes.c_int64),
                                           ctypes.c_size_t]
    lib.axon_start_nrt_profile.restype = ctypes.c_int64
    lib.axon_stop_nrt_profile.argtypes = [ctypes.c_char_p]
    lib.axon_stop_nrt_profile.restype = ctypes.c_int64

    @contextlib.contextmanager
    def _hook(output_dir, device_ids):
        import jax

        jax.devices()
        if device_ids:
            ids = (ctypes.c_int64 * len(device_ids))(*device_ids)
            rc = lib.axon_start_nrt_profile(ids, len(device_ids))
        else:
            rc = lib.axon_start_nrt_profile(None, 0)
        if rc != 0:
            raise RuntimeError(f"axon_start_nrt_profile rc={rc}")
        try:
            yield
        finally:
            n = lib.axon_stop_nrt_profile(str(output_dir).encode())
            print(f"ntff profile: {n} file(s) -> {output_dir}", file=sys.stderr)

    mod = types.ModuleType("antenv.axon_hooks")
    mod.get_axon_ntff_profile_hook = lambda: _hook
    sys.modules["antenv.axon_hooks"] = mod
    import concourse.bass_utils as bass_utils

    bass_utils.upload_artifacts = lambda tmpdir: "local://" + tmpdir


def kernel(content, bank_content, bank_style):
    from concourse.bass_utils import run_bass_kernel_spmd

    content = np.ascontiguousarray(content, dtype=np.float32)
    bank_content = np.ascontiguousarray(bank_content, dtype=np.float32)
    bank_style = np.asarray(bank_style)
    cs = content.reshape(B, D)
    ct = bank_content.reshape(D, M)  # raw row-major reshape, NOT a transpose

    in_maps = []
    for c in range(NCORES):
        lo = c * DSH
        bank_pm = np.ascontiguousarray(
            ct[lo:lo + DSH].reshape(KT, 128, M).transpose(1, 0, 2).astype(BF16))
        in_maps.append({
            "qT": _make_qT(cs, lo),
            "bank": bank_pm,
        })

    nc = _get_nc()
    trace = bool(os.environ.get("BASSKNN_TRACE"))
    kwargs = {}
    if trace:
        _install_ntff_hook()
        kwargs = {"trace": True}
    res = run_bass_kernel_spmd(nc, in_maps, list(range(NCORES)), **kwargs)
    if trace:
        print(f"HW exec time: {res.exec_time_ns} ns")

    dot = np.zeros((B, M), np.float64)
    ssq = np.zeros((M,), np.float64)
    for c in range(NCORES):
        d = res.results[c]["dot_out"].astype(np.float64)
        dot += d[0:64] + d[64:128]
        s = res.results[c]["ssq_out"][0].astype(np.float64)
        ssq += s[:M] + s[M:]
    sim = dot / np.sqrt(ssq)[None, :]  # = cosine * ||cs_b||, per row b

    idx = sim.argmax(axis=1)
    # Exact re-rank of near-ties: any m whose bf16 sim is within
    # RERANK_MARGIN (cosine units) of the row max could be the true winner.
    row_norms = np.sqrt(np.einsum("bd,bd->b", cs, cs, dtype=np.float64))
    col_cache = {}
    for b in range(B):
        thr = RERANK_MARGIN * row_norms[b]
        cands = np.nonzero(sim[b] >= sim[b, idx[b]] - thr)[0]
        if len(cands) <= 1:
            continue
        row = cs[b].astype(np.float64)
        best_m, best_v = -1, -np.inf
        for m in sorted(int(x) for x in cands):
            if m not in col_cache:
                colf = ct[:, m].astype(np.float64)
                col_cache[m] = (colf, np.sqrt(colf @ colf))
            colf, nrm = col_cache[m]
            v = (row @ colf) / nrm
            if v > best_v:  # strict '>' keeps the lowest index on exact ties
                best_v, best_m = v, m
        idx[b] = best_m
    return bank_style[idx]
